# revision 43
# baseline (speedup 1.0000x reference)
"""Trainium2 Bass kernel for a dense transformer block (B=4,T=2048,C=1024,H=16).

Sharding: 8 cores, zero collectives. Core i handles batch i//2 and a balanced
half of the query tokens. The per-core x is HOST-PERMUTED at 128-row tile
granularity so that every core's query tokens sit at fixed tile positions
{0..3, 8..11}; causality under the permutation is enforced by per-core
element-wise multiplicative masks (uniform program, per-core data).

  half=0: perm = [0,1,2,3, 4..7, 12,13,14,15, 8..11]   (queries 0-511, 1536-2047)
  half=1: perm = [4,5,6,7, 0..3, 8,9,10,11, 12..15]    (queries 512-1023, 1024-1535)

Per-core dataflow (all-bf16 matmuls, tokens-on-free-axis):
  LN1 per 128-token tile -> h1 bf16 -> PE transpose (bf16 identity) -> h1T
  quarters; QKV stream per quarter with no DRAM bounce.  kT/qT stacked
  [2*64, tok]; V natural [tok, H*65] with fused ones column so PV also
  yields the softmax denominator.  Scores pre-transposed sT[tk,tq]=K Q^T;
  exp on ACT straight from PSUM with 1/sqrt(hd) scale fused; causality via
  multiplicative masks; PV accumulates in PSUM; normalization = reciprocal
  + gpsimd partition_broadcast + one multiply.  Attention outputs stay in
  SBUF ([64,512] tiles combined to [128,512] via SBUF->SBUF DMA) feeding
  proj directly.  LN2 + PE-transpose h2T fused behind proj.  FFN streams
  w1/w2 once (jb-outer, host-prearranged bf16), accumulating in SBUF.
"""

import sys
import numpy as np

for _p in ("/opt/trn_rl_repo", "/root/.axon_site/_ro/trn_rl_repo"):
    if _p not in sys.path:
        sys.path.append(_p)

import ml_dtypes  # noqa: E402
import concourse.bass as bass  # noqa: E402
import concourse.bacc as bacc  # noqa: E402
import concourse.tile as tile  # noqa: E402
from concourse import mybir  # noqa: E402
from concourse.bass_utils import run_bass_kernel_spmd  # noqa: E402
from concourse.masks import make_identity  # noqa: E402

B, T, C, H, HD = 4, 2048, 1024, 16, 64
NCORES = 8
EPS = 1e-5
F32 = mybir.dt.float32
BF16 = mybir.dt.bfloat16
AF = mybir.ActivationFunctionType
ALU = mybir.AluOpType
SCALE = HD ** -0.5

TILE_PERM = {
    0: [0, 1, 2, 3, 4, 5, 6, 7, 12, 13, 14, 15, 8, 9, 10, 11],
    1: [4, 5, 6, 7, 0, 1, 2, 3, 8, 9, 10, 11, 12, 13, 14, 15],
}

_CACHE = {}


def _emit_body(nc, tc, io, ln1_triv, ln2_triv):
    def pool(name, bufs, space="SBUF"):
        cm = tc.tile_pool(name=name, bufs=bufs, space=space)
        return cm, cm.__enter__()

    # ---------------- long-lived pools (bottom of stack) ----------------
    cm_singles, singles = pool("singles", 1)
    cm_ln, ln_pool = pool("ln", 2)
    cm_stat, stat_pool = pool("stat", 4)
    cm_small, small = pool("small", 2)
    cm_h1, h1_pool = pool("h1", 2)
    cm_kt, kt_pool = pool("kt", 8)
    cm_qt, qt_pool = pool("qt", 8)
    cm_v, v_pool = pool("v", 16)
    cm_masks, masks_pool = pool("masks", 2)
    cm_dram, dram = pool("dram", 1, "DRAM")

    eps_t = singles.tile([128, 1], F32, name="eps")
    nc.vector.memset(eps_t, EPS)
    ident_bf = singles.tile([128, 128], BF16, name="ident_bf")
    make_identity(nc, ident_bf)
    b1t_sb = singles.tile([128, 32], F32, name="b1t_sb")

    def bcast_ap(dram_ap):
        # [1024] dram vector -> [128,1024] partition-broadcast AP
        return bass.AP(
            tensor=dram_ap.tensor,
            offset=dram_ap.offset,
            ap=[[0, 128]] + list(dram_ap.ap),
        )

    bproj_sb = singles.tile([128, 1024], BF16, name="bproj_sb")
    b2_sb = singles.tile([128, 1024], BF16, name="b2_sb")

    g1_sb = bb1_sb = g2_sb = bb2_sb = None
    if not ln1_triv:
        g1_sb = singles.tile([128, 1024], F32, name="g1_sb")
        nc.gpsimd.dma_start(out=g1_sb, in_=bcast_ap(io["ln1_g"]))
        bb1_sb = singles.tile([128, 1024], F32, name="bb1_sb")
        nc.gpsimd.dma_start(out=bb1_sb, in_=bcast_ap(io["ln1_b"]))
    if not ln2_triv:
        g2_sb = singles.tile([128, 1024], F32, name="g2_sb")
        nc.gpsimd.dma_start(out=g2_sb, in_=bcast_ap(io["ln2_g"]))
        bb2_sb = singles.tile([128, 1024], F32, name="bb2_sb")
        nc.gpsimd.dma_start(out=bb2_sb, in_=bcast_ap(io["ln2_b"]))

    kT = [kt_pool.tile([128, 2048], BF16, tag="kt", name="kt") for _ in range(8)]
    qT = [qt_pool.tile([128, 1024], BF16, tag="qt", name="qt") for _ in range(8)]
    Vt = [v_pool.tile([128, 16, 65], BF16, tag="vt", name="vt") for _ in range(16)]
    msk = [masks_pool.tile([128, 8, 512], BF16, tag="m", name="msk")
           for _ in range(2)]

    # ---------------- LayerNorm helpers ----------------
    def ln_apply(xt, out_ap, trivial, g_sb, b_sb):
        st = stat_pool.tile([128, 2, 6], F32, tag="bnst", name="bnst")
        for sg in range(2):
            nc.vector.bn_stats(out=st[:, sg, :], in_=xt[:, sg * 512:(sg + 1) * 512])
        mv = stat_pool.tile([128, 2], F32, tag="bnmv", name="bnmv")
        nc.vector.bn_aggr(out=mv, in_=st)
        std = stat_pool.tile([128, 1], F32, tag="bnsd", name="bnsd")
        nc.scalar.activation(out=std, in_=mv[:, 1:2], func=AF.Sqrt, bias=eps_t,
                             scale=1.0)
        rstd = stat_pool.tile([128, 1], F32, tag="bnrs", name="bnrs")
        nc.vector.reciprocal(out=rstd, in_=std)
        if trivial:
            nc.vector.tensor_scalar(
                out=out_ap, in0=xt, scalar1=mv[:, 0:1], scalar2=rstd,
                op0=ALU.subtract, op1=ALU.mult)
        else:
            tmp = ln_pool.tile([128, 1024], F32, tag="lnx", name="lntmp")
            nc.vector.tensor_scalar(
                out=tmp, in0=xt, scalar1=mv[:, 0:1], scalar2=rstd,
                op0=ALU.subtract, op1=ALU.mult)
            nc.vector.tensor_mul(out=tmp, in0=tmp, in1=g_sb)
            nc.vector.tensor_add(out=out_ap, in0=tmp, in1=b_sb)

    # ---------------- Phase 1: LN1 + PE-transpose + QKV per quarter ---------
    cm_xt, xt_pool = pool("xt", 9)
    cm_h1t, h1t_pool = pool("h1t", 8)
    cm_wk, wk_pool = pool("wk", 1)
    cm_wv, wv_pool = pool("wv", 1)
    cm_wq, wq_pool = pool("wq", 1)
    cm_pst, ps_tr = pool("ps_tr", 4, "PSUM")
    cm_psq, ps_qkv = pool("ps_qkv", 4, "PSUM")

    # Input loads all on the sync queue in need-order: the DMA engine is a
    # serial resource, so arrival order == issue order here.
    xts = [xt_pool.tile([128, 1024], BF16, tag="xt", name="xt")
           for _ in range(16)]
    wkB = wk_pool.tile([128, 8, 1024], BF16, name="wkB")
    wvB = wv_pool.tile([128, 8, 1024], BF16, name="wvB")
    wqB = wq_pool.tile([128, 8, 1024], BF16, name="wqB")
    for t in range(4):
        nc.sync.dma_start(out=xts[t], in_=io["x"][t * 128:(t + 1) * 128, :])
    nc.sync.dma_start(out=wvB[:, 0:4, :], in_=io["wvh"][:, 0:4, :])
    nc.sync.dma_start(out=wvB[:, 4:8, :], in_=io["wvh"][:, 4:8, :])
    for t in range(4, 8):
        nc.sync.dma_start(out=xts[t], in_=io["x"][t * 128:(t + 1) * 128, :])
    nc.sync.dma_start(out=wkB, in_=io["wkh"])
    for t in range(8, 12):
        nc.sync.dma_start(out=xts[t], in_=io["x"][t * 128:(t + 1) * 128, :])
    nc.sync.dma_start(out=wqB, in_=io["wqh"])
    for t in range(12, 16):
        nc.sync.dma_start(out=xts[t], in_=io["x"][t * 128:(t + 1) * 128, :])
    for s in range(2):
        nc.sync.dma_start(out=msk[s], in_=io["masks"][:, s * 8:(s + 1) * 8, :])

    for q in range(4):
        # pair-tiles [128, 2, 512]: adjacent C-chunks share a tile so one
        # ACT copy lands two PE transposes; matmuls slice [:, c % 2, :]
        tiles = [h1t_pool.tile([128, 2, 512], BF16, tag="h1t", name="h1t")
                 for _ in range(4)]
        for tt in range(4):
            t = 4 * q + tt
            ht = h1_pool.tile([128, 1024], BF16, tag="h1", name="h1")
            ln_apply(xts[t], ht, ln1_triv, g1_sb, bb1_sb)
            for cc in range(4):
                ps = ps_tr.tile([128, 256], BF16, tag="tr", name="pstr")
                for k in range(2):
                    c = 2 * cc + k
                    nc.tensor.transpose(out=ps[:, k * 128:(k + 1) * 128],
                                        in_=ht[:, c * 128:(c + 1) * 128],
                                        identity=ident_bf)
                nc.scalar.copy(out=tiles[cc][:, :, tt * 128:(tt + 1) * 128],
                               in_=ps)
            # V for this token tile starts as soon as its transposes land
            for n in range(2):
                ps = ps_qkv.tile([128, 512], F32, tag="q", name="psv")
                for c in range(8):
                    nc.tensor.matmul(
                        out=ps,
                        lhsT=tiles[c // 2][:, c % 2, tt * 128:(tt + 1) * 128],
                        rhs=wvB[:, c, n * 512:(n + 1) * 512],
                        start=(c == 0), stop=(c == 7))
                nc.scalar.copy(
                    out=Vt[t][:, n * 8:(n + 1) * 8, 0:64],
                    in_=ps.rearrange("p (h d) -> p h d", d=64))
                if n == 1:
                    nc.vector.memset(Vt[t][:, :, 64:65], 1.0)
        for p in range(8):
            ps = ps_qkv.tile([128, 512], F32, tag="q", name="psk")
            for c in range(8):
                nc.tensor.matmul(
                    out=ps, lhsT=wkB[:, c, p * 128:(p + 1) * 128],
                    rhs=tiles[c // 2][:, c % 2, :], start=(c == 0),
                    stop=(c == 7))
            nc.vector.tensor_copy(out=kT[p][:, q * 512:(q + 1) * 512], in_=ps)
        if q in (0, 2):
            g = 0 if q == 0 else 1
            for p in range(8):
                ps = ps_qkv.tile([128, 512], F32, tag="q", name="psq")
                for c in range(8):
                    nc.tensor.matmul(
                        out=ps, lhsT=wqB[:, c, p * 128:(p + 1) * 128],
                        rhs=tiles[c // 2][:, c % 2, :], start=(c == 0),
                        stop=(c == 7))
                nc.vector.tensor_copy(out=qT[p][:, g * 512:(g + 1) * 512], in_=ps)

    cm_psq.__exit__(None, None, None)
    cm_pst.__exit__(None, None, None)
    cm_wq.__exit__(None, None, None)
    cm_wv.__exit__(None, None, None)
    cm_wk.__exit__(None, None, None)
    cm_h1t.__exit__(None, None, None)
    cm_xt.__exit__(None, None, None)

    # ---------------- Phase 2: attention ----------------
    cm_ast, ast_pool = pool("ast", 16)
    cm_at, asttmp_pool = pool("asttmp", 2)
    cm_wp, wp_pool = pool("wproj", 1)
    cm_pt, pt_pool = pool("pt", 4)
    cm_pssc, ps_sc = pool("ps_sc", 2, "PSUM")
    cm_pspv, ps_pv = pool("ps_pv", 4, "PSUM")

    wpB = wp_pool.tile([128, 8, 1024], BF16, name="wpB")
    nc.sync.dma_start(out=wpB, in_=io["wph"])
    nc.gpsimd.dma_start(out=bproj_sb, in_=bcast_ap(io["b_proj"]))
    nc.gpsimd.dma_start(out=b2_sb, in_=bcast_ap(io["b2"]))
    nc.gpsimd.dma_start(out=b1t_sb, in_=io["b1t"])

    ast = {}  # (s, hp) -> [128, 512] bf16, channels 2hp|2hp+1 stacked
    for s in range(2):
        ntk = 8 if s == 0 else 16
        for hp in range(8):
            pva = [ps_pv.tile([128, 512], F32, tag="pv", name="pv")
                   for _ in range(2)]
            pts = {}
            for tkt in range(ntk):
                ps = ps_sc.tile([128, 2, 512], F32, tag="sc", name="sc")
                for e in range(2):
                    nc.tensor.matmul(
                        out=ps[:, e, :],
                        lhsT=kT[hp][e * 64:(e + 1) * 64,
                                    tkt * 128:(tkt + 1) * 128],
                        rhs=qT[hp][e * 64:(e + 1) * 64,
                                   s * 512:(s + 1) * 512],
                        start=True, stop=True)
                pt = pt_pool.tile([128, 2, 512], BF16, tag="pt", name="pt")
                pts[tkt] = pt
                nc.scalar.activation(
                    out=pt.rearrange("p a b -> p (a b)"),
                    in_=ps.rearrange("p a b -> p (a b)"),
                    func=AF.Exp, scale=SCALE)
                if s == 0 or tkt >= 8:
                    mi = tkt if s == 0 else tkt - 8
                    mrow = msk[s][:, mi, :]
                    mb = bass.AP(
                        tensor=mrow.tensor,
                        offset=mrow.offset,
                        ap=[list(mrow.ap[0]), [0, 2]] + list(mrow.ap[1:]))
                    nc.vector.tensor_mul(out=pt, in0=pt, in1=mb)
                if tkt >= 2:
                    prev = pts.pop(tkt - 2)
                    for e in range(2):
                        nc.tensor.matmul(
                            out=pva[e][0:65, :],
                            lhsT=Vt[tkt - 2][:, 2 * hp + e, :],
                            rhs=prev[:, e, :],
                            start=(tkt - 2 == 0), stop=False)
            for tl in (ntk - 2, ntk - 1):
                last = pts.pop(tl)
                for e in range(2):
                    nc.tensor.matmul(
                        out=pva[e][0:65, :],
                        lhsT=Vt[tl][:, 2 * hp + e, :],
                        rhs=last[:, e, :],
                        start=False, stop=(tl == ntk - 1))
            a = ast_pool.tile([128, 512], BF16, tag="ast", name="ast")
            ast[(s, hp)] = a
            for e in range(2):
                rec = small.tile([1, 512], F32, tag="rec", name="rec")
                nc.vector.reciprocal(out=rec, in_=pva[e][64:65, :])
                bc = small.tile([64, 512], F32, tag="bc", name="bc")
                nc.gpsimd.partition_broadcast(out_ap=bc, in_ap=rec)
                if e == 0:
                    nc.vector.tensor_mul(out=a[0:64, :], in0=pva[e][0:64, :],
                                         in1=bc)
                else:
                    tmp = asttmp_pool.tile([64, 512], BF16, tag="att",
                                           name="asttmp")
                    nc.vector.tensor_mul(out=tmp, in0=pva[e][0:64, :], in1=bc)
                    nc.sync.dma_start(out=a[64:128, :], in_=tmp)

    cm_pspv.__exit__(None, None, None)
    cm_pssc.__exit__(None, None, None)
    cm_pt.__exit__(None, None, None)

    # ---------------- Phase 3: proj + LN2 + h2T ----------------
    cm_w1, w1_pool = pool("w1b", 1)
    cm_w2, w2_pool = pool("w2b", 2)
    w1b0 = w1_pool.tile([128, 8, 1024], BF16, tag="w1", name="w1b")
    nc.gpsimd.dma_start(out=w1b0, in_=io["w1h"][0])
    w2b0 = w2_pool.tile([128, 8, 1024], BF16, tag="w2", name="w2b")
    nc.gpsimd.dma_start(out=w2b0, in_=io["w2h"][0])
    cm_pspr, ps_pr = pool("ps_pr", 4, "PSUM")
    cm_pst2, ps_tr2 = pool("ps_tr2", 4, "PSUM")

    h2T = [qt_pool.tile([128, 1024], BF16, tag="qt", name="h2t")
           for _ in range(8)]
    # x2t tiles live in kT's dead buffers and double as the FFN accumulators
    x2s = []
    for t in range(8):
        s, tt = (0, t) if t < 4 else (1, t - 4)
        tp = t if t < 4 else t + 4
        xh = ln_pool.tile([128, 1024], BF16, tag="lnx", name="xh2")
        nc.sync.dma_start(out=xh, in_=io["x"][tp * 128:(tp + 1) * 128, :])
        x2t = kt_pool.tile([128, 1024], F32, tag="kt", name="x2t")
        x2s.append(x2t)
        for n in range(2):
            ps = ps_pr.tile([128, 512], F32, tag="pr", name="pspr")
            for c in range(8):
                nc.tensor.matmul(
                    out=ps, lhsT=ast[(s, c)][:, tt * 128:(tt + 1) * 128],
                    rhs=wpB[:, c, n * 512:(n + 1) * 512],
                    start=(c == 0), stop=(c == 7))
            sl = np.s_[:, n * 512:(n + 1) * 512]
            nc.vector.tensor_add(out=x2t[sl], in0=ps, in1=xh[sl])
            nc.vector.tensor_add(out=x2t[sl], in0=x2t[sl], in1=bproj_sb[sl])
        h2 = h1_pool.tile([128, 1024], BF16, tag="h1", name="h2")
        ln_apply(x2t, h2, ln2_triv, g2_sb, bb2_sb)
        for c in range(8):
            pst = ps_tr2.tile([128, 128], BF16, tag="tr2", name="pst2")
            nc.tensor.transpose(out=pst, in_=h2[:, c * 128:(c + 1) * 128],
                                identity=ident_bf)
            nc.scalar.copy(out=h2T[c][:, t * 128:(t + 1) * 128], in_=pst)

    cm_pst2.__exit__(None, None, None)
    cm_pspr.__exit__(None, None, None)

    # ---------------- Phase 4: FFN (jb-outer, weights streamed once) --------
    cm_rl, relu_pool = pool("relu", 1)
    cm_psf, ps_f = pool("ps_f", 4, "PSUM")

    oacc = x2s  # accumulate the FFN output directly onto the residual stream
    for jb in range(4):
        if jb == 0:
            w1b, w2b = w1b0, w2b0
        else:
            w1b = w1_pool.tile([128, 8, 1024], BF16, tag="w1", name="w1b")
            nc.gpsimd.dma_start(out=w1b, in_=io["w1h"][jb])
            w2b = w2_pool.tile([128, 8, 1024], BF16, tag="w2", name="w2b")
            nc.gpsimd.dma_start(out=w2b, in_=io["w2h"][jb])
        for pas in range(2):
            relu_b = relu_pool.tile([128, 8, 512], BF16, tag="rl", name="rl")
            for j in range(8):
                ps = ps_f.tile([128, 512], F32, tag="f", name="psf1")
                for c in range(8):
                    nc.tensor.matmul(
                        out=ps, lhsT=w1b[:, c, j * 128:(j + 1) * 128],
                        rhs=h2T[c][:, pas * 512:(pas + 1) * 512],
                        start=(c == 0), stop=(c == 7))
                nc.scalar.activation(
                    out=relu_b[:, j, :], in_=ps, func=AF.Relu,
                    bias=b1t_sb[:, jb * 8 + j:jb * 8 + j + 1], scale=1.0)
            for tl in range(4):
                tg = pas * 4 + tl
                for n in range(2):
                    ps = ps_f.tile([128, 512], F32, tag="f", name="psf2")
                    for j in range(8):
                        nc.tensor.matmul(
                            out=ps,
                            lhsT=relu_b[:, j, tl * 128:(tl + 1) * 128],
                            rhs=w2b[:, j, n * 512:(n + 1) * 512],
                            start=(j == 0), stop=(j == 7))
                    sl = np.s_[:, n * 512:(n + 1) * 512]
                    if jb == 0:
                        nc.vector.tensor_add(out=oacc[tg][sl],
                                             in0=oacc[tg][sl], in1=ps)
                        nc.vector.tensor_add(out=oacc[tg][sl],
                                             in0=oacc[tg][sl], in1=b2_sb[sl])
                    else:
                        nc.vector.tensor_add(out=oacc[tg][sl],
                                             in0=oacc[tg][sl], in1=ps)
                    if jb == 3:
                        nc.sync.dma_start(
                            out=io["out"][tg * 128:(tg + 1) * 128,
                                          n * 512:(n + 1) * 512],
                            in_=oacc[tg][sl])

    cm_psf.__exit__(None, None, None)
    cm_rl.__exit__(None, None, None)
    cm_w2.__exit__(None, None, None)
    cm_w1.__exit__(None, None, None)
    cm_wp.__exit__(None, None, None)
    cm_at.__exit__(None, None, None)
    cm_ast.__exit__(None, None, None)
    cm_dram.__exit__(None, None, None)
    cm_masks.__exit__(None, None, None)
    cm_v.__exit__(None, None, None)
    cm_qt.__exit__(None, None, None)
    cm_kt.__exit__(None, None, None)
    cm_h1.__exit__(None, None, None)
    cm_small.__exit__(None, None, None)
    cm_stat.__exit__(None, None, None)
    cm_ln.__exit__(None, None, None)
    cm_singles.__exit__(None, None, None)


def build(ln1_triv=True, ln2_triv=True):
    key = (ln1_triv, ln2_triv)
    if key in _CACHE:
        return _CACHE[key]
    nc = bacc.Bacc("TRN2", target_bir_lowering=False, debug=False,
                   num_devices=NCORES)
    io = {}

    def din(name, shape, dt):
        io[name] = nc.dram_tensor(name, list(shape), dt, kind="ExternalInput").ap()

    din("x", (2048, 1024), BF16)
    din("wkh", (128, 8, 1024), BF16)
    din("wvh", (128, 8, 1024), BF16)
    din("wqh", (128, 8, 1024), BF16)
    din("wph", (128, 8, 1024), BF16)
    din("b_proj", (1024,), BF16)
    din("w1h", (4, 128, 8, 1024), BF16)
    din("b1t", (128, 32), F32)
    din("w2h", (4, 128, 8, 1024), BF16)
    din("b2", (1024,), BF16)
    din("masks", (128, 16, 512), BF16)
    if not ln1_triv:
        din("ln1_g", (1024,), F32)
        din("ln1_b", (1024,), F32)
    if not ln2_triv:
        din("ln2_g", (1024,), F32)
        din("ln2_b", (1024,), F32)
    io["out"] = nc.dram_tensor("out", [1024, 1024], F32, kind="ExternalOutput").ap()

    with tile.TileContext(nc) as tc:
        _emit_body(nc, tc, io, ln1_triv, ln2_triv)
    nc.compile()
    _CACHE[key] = (nc, io)
    return nc, io


def _chunks(half):
    # original query-chunk bases (chunk A, chunk B) — rows 0-511 / 512-1023
    # of the per-core output
    if half == 0:
        return (0, 1536)
    return (512, 1024)


def _make_masks(half):
    """[128, 16, 512] bf16; m 0-7 = slot0 (perm k-tiles 0-7 vs slot0
    queries), m 8-15 = slot1 (perm k-tiles 8-15 vs slot1 queries)."""
    P = np.array(TILE_PERM[half])
    tk = np.arange(128)
    ko = P[:, None] * 128 + tk[None, :]                    # [16, 128] orig k idx
    tq = np.arange(512)
    qo0 = (P[tq // 128 + 0] * 128 + tq % 128)              # slot0 queries
    qo1 = (P[tq // 128 + 8] * 128 + tq % 128)              # slot1 queries
    # slot1 skips masks for perm k-tiles 0-7: must be fully allowed
    assert ko[0:8].max() < qo1.min()
    out = np.zeros((128, 16, 512), np.float32)
    for m in range(8):
        out[:, m, :] = (ko[m][:, None] <= qo0[None, :])
    for m in range(8, 16):
        out[:, m, :] = (ko[m][:, None] <= qo1[None, :])
    return out.astype(ml_dtypes.bfloat16)


def _wprep(w):
    # [C, N] f32 -> [128, 8, 1024] bf16 with w_out[p, c, n] = w[c*128+p, n]
    return np.ascontiguousarray(
        w.reshape(8, 128, w.shape[1]).transpose(1, 0, 2)
    ).astype(ml_dtypes.bfloat16)


def _prep_common(inp, ln1_triv, ln2_triv):
    bf = ml_dtypes.bfloat16
    wq_f = np.ascontiguousarray(inp["wq"].transpose(1, 0, 2).reshape(C, C))
    wk_f = np.ascontiguousarray(inp["wk"].transpose(1, 0, 2).reshape(C, C))
    wv_f = np.ascontiguousarray(inp["wv"].transpose(1, 0, 2).reshape(C, C))
    w1 = np.asarray(inp["w1"])
    w2 = np.asarray(inp["w2"])
    # w1h[jb, p, c, n] = w1[c*128+p, jb*1024+n]
    w1h = np.ascontiguousarray(
        w1.reshape(8, 128, 4, 1024).transpose(2, 1, 0, 3)).astype(bf)
    # w2h[jb, p, j, n] = w2[jb*1024 + j*128 + p, n]
    w2h = np.ascontiguousarray(
        w2.reshape(4, 8, 128, 1024).transpose(0, 2, 1, 3)).astype(bf)
    b1t = np.ascontiguousarray(inp["b1"].reshape(32, 128).T).astype(np.float32)
    common = {
        "wqh": _wprep(wq_f),
        "wkh": _wprep(wk_f),
        "wvh": _wprep(wv_f),
        "wph": _wprep(np.asarray(inp["w_proj"])),
        "b_proj": inp["b_proj"].astype(bf),
        "w1h": w1h,
        "b1t": b1t,
        "w2h": w2h,
        "b2": inp["b2"].astype(bf),
    }
    if not ln1_triv:
        common["ln1_g"] = inp["ln1_g"].astype(np.float32)
        common["ln1_b"] = inp["ln1_b"].astype(np.float32)
    if not ln2_triv:
        common["ln2_g"] = inp["ln2_g"].astype(np.float32)
        common["ln2_b"] = inp["ln2_b"].astype(np.float32)
    return common


def make_in_maps(inputs):
    inp = {k: np.asarray(v) for k, v in inputs.items()}
    x = inp["x"].astype(np.float32)
    ln1_triv = bool(np.all(inp["ln1_g"] == 1.0) and np.all(inp["ln1_b"] == 0.0))
    ln2_triv = bool(np.all(inp["ln2_g"] == 1.0) and np.all(inp["ln2_b"] == 0.0))
    common = _prep_common(inp, ln1_triv, ln2_triv)
    in_maps = []
    for i in range(NCORES):
        b, half = i // 2, i % 2
        P = TILE_PERM[half]
        xp = np.ascontiguousarray(
            x[b].reshape(16, 128, C)[P].reshape(2048, C)).astype(
                ml_dtypes.bfloat16)
        m = dict(common)
        m["x"] = xp
        m["masks"] = _make_masks(half)
        in_maps.append(m)
    return in_maps, ln1_triv, ln2_triv


def assemble(results):
    out = np.empty((B, T, C), np.float32)
    for i in range(NCORES):
        b, half = i // 2, i % 2
        qa, qb = _chunks(half)
        o = results[i]["out"]
        out[b, qa:qa + 512] = o[:512]
        out[b, qb:qb + 512] = o[512:]
    return out


def kernel(**inputs):
    in_maps, l1, l2 = make_in_maps(inputs)
    nc, io = build(l1, l2)
    res = run_bass_kernel_spmd(nc, in_maps, list(range(NCORES)))
    return assemble(res.results)


if __name__ == "__main__":
    build()
    print("build ok")



# revision 44
# speedup vs baseline: 1.0042x; 1.0042x over previous
"""Trainium2 Bass kernel for a dense transformer block (B=4,T=2048,C=1024,H=16).

Sharding: 8 cores, zero collectives. Core i handles batch i//2 and a balanced
half of the query tokens. The per-core x is HOST-PERMUTED at 128-row tile
granularity so that every core's query tokens sit at fixed tile positions
{0..3, 8..11}; causality under the permutation is enforced by per-core
element-wise multiplicative masks (uniform program, per-core data).

  half=0: perm = [0,1,2,3, 4..7, 12,13,14,15, 8..11]   (queries 0-511, 1536-2047)
  half=1: perm = [4,5,6,7, 0..3, 8,9,10,11, 12..15]    (queries 512-1023, 1024-1535)

Per-core dataflow (all-bf16 matmuls, tokens-on-free-axis):
  LN1 per 128-token tile -> h1 bf16 -> PE transpose (bf16 identity) -> h1T
  quarters; QKV stream per quarter with no DRAM bounce.  kT/qT stacked
  [2*64, tok]; V natural [tok, H*65] with fused ones column so PV also
  yields the softmax denominator.  Scores pre-transposed sT[tk,tq]=K Q^T;
  exp on ACT straight from PSUM with 1/sqrt(hd) scale fused; causality via
  multiplicative masks; PV accumulates in PSUM; normalization = reciprocal
  + gpsimd partition_broadcast + one multiply.  Attention outputs stay in
  SBUF ([64,512] tiles combined to [128,512] via SBUF->SBUF DMA) feeding
  proj directly.  LN2 + PE-transpose h2T fused behind proj.  x2 stays
  SBUF-resident (kT's dead buffers) and doubles as the FFN accumulator --
  no DRAM bounce.  FFN streams w1/w2 once (jb-outer, host-prearranged
  bf16, w2 double-buffered so jb-boundary reloads hide under compute),
  accumulating onto x2 in SBUF.  Transpose PSUM->SBUF copies are packed
  two-per-ACT-instruction; V is emitted per token tile right after its
  transposes so PE starts early.
"""

import sys
import numpy as np

for _p in ("/opt/trn_rl_repo", "/root/.axon_site/_ro/trn_rl_repo"):
    if _p not in sys.path:
        sys.path.append(_p)

import ml_dtypes  # noqa: E402
import concourse.bass as bass  # noqa: E402
import concourse.bacc as bacc  # noqa: E402
import concourse.tile as tile  # noqa: E402
from concourse import mybir  # noqa: E402
from concourse.bass_utils import run_bass_kernel_spmd  # noqa: E402
from concourse.masks import make_identity  # noqa: E402

B, T, C, H, HD = 4, 2048, 1024, 16, 64
NCORES = 8
EPS = 1e-5
F32 = mybir.dt.float32
BF16 = mybir.dt.bfloat16
AF = mybir.ActivationFunctionType
ALU = mybir.AluOpType
SCALE = HD ** -0.5

TILE_PERM = {
    0: [0, 1, 2, 3, 4, 5, 6, 7, 12, 13, 14, 15, 8, 9, 10, 11],
    1: [4, 5, 6, 7, 0, 1, 2, 3, 8, 9, 10, 11, 12, 13, 14, 15],
}

_CACHE = {}


def _emit_body(nc, tc, io, ln1_triv, ln2_triv):
    def pool(name, bufs, space="SBUF"):
        cm = tc.tile_pool(name=name, bufs=bufs, space=space)
        return cm, cm.__enter__()

    # ---------------- long-lived pools (bottom of stack) ----------------
    cm_singles, singles = pool("singles", 1)
    cm_ln, ln_pool = pool("ln", 2)
    cm_stat, stat_pool = pool("stat", 4)
    cm_small, small = pool("small", 2)
    cm_h1, h1_pool = pool("h1", 2)
    cm_kt, kt_pool = pool("kt", 8)
    cm_qt, qt_pool = pool("qt", 8)
    cm_v, v_pool = pool("v", 16)
    cm_masks, masks_pool = pool("masks", 2)
    cm_dram, dram = pool("dram", 1, "DRAM")

    eps_t = singles.tile([128, 1], F32, name="eps")
    nc.vector.memset(eps_t, EPS)
    ident_bf = singles.tile([128, 128], BF16, name="ident_bf")
    make_identity(nc, ident_bf)
    b1t_sb = singles.tile([128, 32], F32, name="b1t_sb")

    def bcast_ap(dram_ap):
        # [1024] dram vector -> [128,1024] partition-broadcast AP
        return bass.AP(
            tensor=dram_ap.tensor,
            offset=dram_ap.offset,
            ap=[[0, 128]] + list(dram_ap.ap),
        )

    bproj_sb = singles.tile([128, 1024], BF16, name="bproj_sb")
    b2_sb = singles.tile([128, 1024], BF16, name="b2_sb")

    g1_sb = bb1_sb = g2_sb = bb2_sb = None
    if not ln1_triv:
        g1_sb = singles.tile([128, 1024], F32, name="g1_sb")
        nc.gpsimd.dma_start(out=g1_sb, in_=bcast_ap(io["ln1_g"]))
        bb1_sb = singles.tile([128, 1024], F32, name="bb1_sb")
        nc.gpsimd.dma_start(out=bb1_sb, in_=bcast_ap(io["ln1_b"]))
    if not ln2_triv:
        g2_sb = singles.tile([128, 1024], F32, name="g2_sb")
        nc.gpsimd.dma_start(out=g2_sb, in_=bcast_ap(io["ln2_g"]))
        bb2_sb = singles.tile([128, 1024], F32, name="bb2_sb")
        nc.gpsimd.dma_start(out=bb2_sb, in_=bcast_ap(io["ln2_b"]))

    kT = [kt_pool.tile([128, 2048], BF16, tag="kt", name="kt") for _ in range(8)]
    qT = [qt_pool.tile([128, 1024], BF16, tag="qt", name="qt") for _ in range(8)]
    Vt = [v_pool.tile([128, 16, 65], BF16, tag="vt", name="vt") for _ in range(16)]
    msk = [masks_pool.tile([128, 8, 512], BF16, tag="m", name="msk")
           for _ in range(2)]

    # ---------------- LayerNorm helpers ----------------
    def ln_apply(xt, out_ap, trivial, g_sb, b_sb):
        st = stat_pool.tile([128, 2, 6], F32, tag="bnst", name="bnst")
        for sg in range(2):
            nc.vector.bn_stats(out=st[:, sg, :], in_=xt[:, sg * 512:(sg + 1) * 512])
        mv = stat_pool.tile([128, 2], F32, tag="bnmv", name="bnmv")
        nc.vector.bn_aggr(out=mv, in_=st)
        std = stat_pool.tile([128, 1], F32, tag="bnsd", name="bnsd")
        nc.scalar.activation(out=std, in_=mv[:, 1:2], func=AF.Sqrt, bias=eps_t,
                             scale=1.0)
        rstd = stat_pool.tile([128, 1], F32, tag="bnrs", name="bnrs")
        nc.vector.reciprocal(out=rstd, in_=std)
        if trivial:
            nc.vector.tensor_scalar(
                out=out_ap, in0=xt, scalar1=mv[:, 0:1], scalar2=rstd,
                op0=ALU.subtract, op1=ALU.mult)
        else:
            tmp = ln_pool.tile([128, 1024], F32, tag="lnx", name="lntmp")
            nc.vector.tensor_scalar(
                out=tmp, in0=xt, scalar1=mv[:, 0:1], scalar2=rstd,
                op0=ALU.subtract, op1=ALU.mult)
            nc.vector.tensor_mul(out=tmp, in0=tmp, in1=g_sb)
            nc.vector.tensor_add(out=out_ap, in0=tmp, in1=b_sb)

    # ---------------- Phase 1: LN1 + PE-transpose + QKV per quarter ---------
    cm_xt, xt_pool = pool("xt", 9)
    cm_h1t, h1t_pool = pool("h1t", 8)
    cm_wk, wk_pool = pool("wk", 1)
    cm_wv, wv_pool = pool("wv", 1)
    cm_wq, wq_pool = pool("wq", 1)
    cm_pst, ps_tr = pool("ps_tr", 4, "PSUM")
    cm_psq, ps_qkv = pool("ps_qkv", 4, "PSUM")

    # Input loads all on the sync queue in need-order: the DMA engine is a
    # serial resource, so arrival order == issue order here.
    xts = [xt_pool.tile([128, 1024], BF16, tag="xt", name="xt")
           for _ in range(16)]
    wkB = wk_pool.tile([128, 8, 1024], BF16, name="wkB")
    wvB = wv_pool.tile([128, 8, 1024], BF16, name="wvB")
    wqB = wq_pool.tile([128, 8, 1024], BF16, name="wqB")
    for t in range(4):
        nc.sync.dma_start(out=xts[t], in_=io["x"][t * 128:(t + 1) * 128, :])
    nc.sync.dma_start(out=wvB[:, 0:4, :], in_=io["wvh"][:, 0:4, :])
    nc.sync.dma_start(out=wvB[:, 4:8, :], in_=io["wvh"][:, 4:8, :])
    for t in range(4, 8):
        nc.sync.dma_start(out=xts[t], in_=io["x"][t * 128:(t + 1) * 128, :])
    nc.sync.dma_start(out=wkB, in_=io["wkh"])
    for t in range(8, 12):
        nc.sync.dma_start(out=xts[t], in_=io["x"][t * 128:(t + 1) * 128, :])
    nc.sync.dma_start(out=wqB, in_=io["wqh"])
    for t in range(12, 16):
        nc.sync.dma_start(out=xts[t], in_=io["x"][t * 128:(t + 1) * 128, :])
    for s in range(2):
        nc.sync.dma_start(out=msk[s], in_=io["masks"][:, s * 8:(s + 1) * 8, :])

    for q in range(4):
        # pair-tiles [128, 2, 512]: adjacent C-chunks share a tile so one
        # ACT copy lands two PE transposes; matmuls slice [:, c % 2, :]
        tiles = [h1t_pool.tile([128, 2, 512], BF16, tag="h1t", name="h1t")
                 for _ in range(4)]
        for tt in range(4):
            t = 4 * q + tt
            ht = h1_pool.tile([128, 1024], BF16, tag="h1", name="h1")
            ln_apply(xts[t], ht, ln1_triv, g1_sb, bb1_sb)
            for cc in range(4):
                ps = ps_tr.tile([128, 256], BF16, tag="tr", name="pstr")
                for k in range(2):
                    c = 2 * cc + k
                    nc.tensor.transpose(out=ps[:, k * 128:(k + 1) * 128],
                                        in_=ht[:, c * 128:(c + 1) * 128],
                                        identity=ident_bf)
                nc.scalar.copy(out=tiles[cc][:, :, tt * 128:(tt + 1) * 128],
                               in_=ps)
            # V for this token tile starts as soon as its transposes land
            for n in range(2):
                ps = ps_qkv.tile([128, 512], F32, tag="q", name="psv")
                for c in range(8):
                    nc.tensor.matmul(
                        out=ps,
                        lhsT=tiles[c // 2][:, c % 2, tt * 128:(tt + 1) * 128],
                        rhs=wvB[:, c, n * 512:(n + 1) * 512],
                        start=(c == 0), stop=(c == 7))
                nc.scalar.copy(
                    out=Vt[t][:, n * 8:(n + 1) * 8, 0:64],
                    in_=ps.rearrange("p (h d) -> p h d", d=64))
                if n == 1:
                    nc.vector.memset(Vt[t][:, :, 64:65], 1.0)
        for p in range(8):
            ps = ps_qkv.tile([128, 512], F32, tag="q", name="psk")
            for c in range(8):
                nc.tensor.matmul(
                    out=ps, lhsT=wkB[:, c, p * 128:(p + 1) * 128],
                    rhs=tiles[c // 2][:, c % 2, :], start=(c == 0),
                    stop=(c == 7))
            nc.vector.tensor_copy(out=kT[p][:, q * 512:(q + 1) * 512], in_=ps)
        if q in (0, 2):
            g = 0 if q == 0 else 1
            for p in range(8):
                ps = ps_qkv.tile([128, 512], F32, tag="q", name="psq")
                for c in range(8):
                    nc.tensor.matmul(
                        out=ps, lhsT=wqB[:, c, p * 128:(p + 1) * 128],
                        rhs=tiles[c // 2][:, c % 2, :], start=(c == 0),
                        stop=(c == 7))
                nc.vector.tensor_copy(out=qT[p][:, g * 512:(g + 1) * 512], in_=ps)

    cm_psq.__exit__(None, None, None)
    cm_pst.__exit__(None, None, None)
    cm_wq.__exit__(None, None, None)
    cm_wv.__exit__(None, None, None)
    cm_wk.__exit__(None, None, None)
    cm_h1t.__exit__(None, None, None)
    cm_xt.__exit__(None, None, None)

    # ---------------- Phase 2: attention ----------------
    cm_ast, ast_pool = pool("ast", 16)
    cm_at, asttmp_pool = pool("asttmp", 2)
    cm_wp, wp_pool = pool("wproj", 1)
    cm_pt, pt_pool = pool("pt", 4)
    cm_pssc, ps_sc = pool("ps_sc", 2, "PSUM")
    cm_pspv, ps_pv = pool("ps_pv", 4, "PSUM")

    wpB = wp_pool.tile([128, 8, 1024], BF16, name="wpB")
    nc.sync.dma_start(out=wpB, in_=io["wph"])
    nc.gpsimd.dma_start(out=bproj_sb, in_=bcast_ap(io["b_proj"]))
    nc.gpsimd.dma_start(out=b2_sb, in_=bcast_ap(io["b2"]))
    nc.gpsimd.dma_start(out=b1t_sb, in_=io["b1t"])

    ast = {}  # (s, hp) -> [128, 512] bf16, channels 2hp|2hp+1 stacked
    for s in range(2):
        ntk = 8 if s == 0 else 16
        for hp in range(8):
            pva = [ps_pv.tile([128, 512], F32, tag="pv", name="pv")
                   for _ in range(2)]
            pts = {}
            for tkt in range(ntk):
                ps = ps_sc.tile([128, 2, 512], F32, tag="sc", name="sc")
                for e in range(2):
                    nc.tensor.matmul(
                        out=ps[:, e, :],
                        lhsT=kT[hp][e * 64:(e + 1) * 64,
                                    tkt * 128:(tkt + 1) * 128],
                        rhs=qT[hp][e * 64:(e + 1) * 64,
                                   s * 512:(s + 1) * 512],
                        start=True, stop=True)
                pt = pt_pool.tile([128, 2, 512], BF16, tag="pt", name="pt")
                pts[tkt] = pt
                nc.scalar.activation(
                    out=pt.rearrange("p a b -> p (a b)"),
                    in_=ps.rearrange("p a b -> p (a b)"),
                    func=AF.Exp, scale=SCALE)
                if s == 0 or tkt >= 8:
                    mi = tkt if s == 0 else tkt - 8
                    mrow = msk[s][:, mi, :]
                    mb = bass.AP(
                        tensor=mrow.tensor,
                        offset=mrow.offset,
                        ap=[list(mrow.ap[0]), [0, 2]] + list(mrow.ap[1:]))
                    nc.vector.tensor_mul(out=pt, in0=pt, in1=mb)
                if tkt >= 2:
                    prev = pts.pop(tkt - 2)
                    for e in range(2):
                        nc.tensor.matmul(
                            out=pva[e][0:65, :],
                            lhsT=Vt[tkt - 2][:, 2 * hp + e, :],
                            rhs=prev[:, e, :],
                            start=(tkt - 2 == 0), stop=False)
            for tl in (ntk - 2, ntk - 1):
                last = pts.pop(tl)
                for e in range(2):
                    nc.tensor.matmul(
                        out=pva[e][0:65, :],
                        lhsT=Vt[tl][:, 2 * hp + e, :],
                        rhs=last[:, e, :],
                        start=False, stop=(tl == ntk - 1))
            a = ast_pool.tile([128, 512], BF16, tag="ast", name="ast")
            ast[(s, hp)] = a
            for e in range(2):
                rec = small.tile([1, 512], F32, tag="rec", name="rec")
                nc.vector.reciprocal(out=rec, in_=pva[e][64:65, :])
                bc = small.tile([64, 512], F32, tag="bc", name="bc")
                nc.gpsimd.partition_broadcast(out_ap=bc, in_ap=rec)
                if e == 0:
                    nc.vector.tensor_mul(out=a[0:64, :], in0=pva[e][0:64, :],
                                         in1=bc)
                else:
                    tmp = asttmp_pool.tile([64, 512], BF16, tag="att",
                                           name="asttmp")
                    nc.vector.tensor_mul(out=tmp, in0=pva[e][0:64, :], in1=bc)
                    nc.sync.dma_start(out=a[64:128, :], in_=tmp)

    cm_pspv.__exit__(None, None, None)
    cm_pssc.__exit__(None, None, None)
    cm_pt.__exit__(None, None, None)

    # ---------------- Phase 3: proj + LN2 + h2T ----------------
    cm_w1, w1_pool = pool("w1b", 1)
    cm_w2, w2_pool = pool("w2b", 2)
    w1b0 = w1_pool.tile([128, 8, 1024], BF16, tag="w1", name="w1b")
    nc.gpsimd.dma_start(out=w1b0, in_=io["w1h"][0])
    w2b0 = w2_pool.tile([128, 8, 1024], BF16, tag="w2", name="w2b")
    nc.gpsimd.dma_start(out=w2b0, in_=io["w2h"][0])
    cm_pspr, ps_pr = pool("ps_pr", 4, "PSUM")
    cm_pst2, ps_tr2 = pool("ps_tr2", 4, "PSUM")

    h2T = [qt_pool.tile([128, 1024], BF16, tag="qt", name="h2t")
           for _ in range(8)]
    # x2t tiles live in kT's dead buffers and double as the FFN accumulators
    x2s = []
    for t in range(8):
        s, tt = (0, t) if t < 4 else (1, t - 4)
        tp = t if t < 4 else t + 4
        xh = ln_pool.tile([128, 1024], BF16, tag="lnx", name="xh2")
        nc.sync.dma_start(out=xh, in_=io["x"][tp * 128:(tp + 1) * 128, :])
        x2t = kt_pool.tile([128, 1024], F32, tag="kt", name="x2t")
        x2s.append(x2t)
        for n in range(2):
            ps = ps_pr.tile([128, 512], F32, tag="pr", name="pspr")
            for c in range(8):
                nc.tensor.matmul(
                    out=ps, lhsT=ast[(s, c)][:, tt * 128:(tt + 1) * 128],
                    rhs=wpB[:, c, n * 512:(n + 1) * 512],
                    start=(c == 0), stop=(c == 7))
            sl = np.s_[:, n * 512:(n + 1) * 512]
            nc.vector.tensor_add(out=x2t[sl], in0=ps, in1=xh[sl])
            nc.vector.tensor_add(out=x2t[sl], in0=x2t[sl], in1=bproj_sb[sl])
        h2 = h1_pool.tile([128, 1024], BF16, tag="h1", name="h2")
        ln_apply(x2t, h2, ln2_triv, g2_sb, bb2_sb)
        for c in range(8):
            pst = ps_tr2.tile([128, 128], BF16, tag="tr2", name="pst2")
            nc.tensor.transpose(out=pst, in_=h2[:, c * 128:(c + 1) * 128],
                                identity=ident_bf)
            nc.scalar.copy(out=h2T[c][:, t * 128:(t + 1) * 128], in_=pst)

    cm_pst2.__exit__(None, None, None)
    cm_pspr.__exit__(None, None, None)

    # ---------------- Phase 4: FFN (jb-outer, weights streamed once) --------
    cm_rl, relu_pool = pool("relu", 1)
    cm_psf, ps_f = pool("ps_f", 4, "PSUM")

    oacc = x2s  # accumulate the FFN output directly onto the residual stream
    for jb in range(4):
        if jb == 0:
            w1b, w2b = w1b0, w2b0
        else:
            w1b = w1_pool.tile([128, 8, 1024], BF16, tag="w1", name="w1b")
            nc.gpsimd.dma_start(out=w1b, in_=io["w1h"][jb])
            w2b = w2_pool.tile([128, 8, 1024], BF16, tag="w2", name="w2b")
            nc.gpsimd.dma_start(out=w2b, in_=io["w2h"][jb])
        for pas in range(2):
            relu_b = relu_pool.tile([128, 8, 512], BF16, tag="rl", name="rl")
            for j in range(8):
                ps = ps_f.tile([128, 512], F32, tag="f", name="psf1")
                for c in range(8):
                    nc.tensor.matmul(
                        out=ps, lhsT=w1b[:, c, j * 128:(j + 1) * 128],
                        rhs=h2T[c][:, pas * 512:(pas + 1) * 512],
                        start=(c == 0), stop=(c == 7))
                nc.scalar.activation(
                    out=relu_b[:, j, :], in_=ps, func=AF.Relu,
                    bias=b1t_sb[:, jb * 8 + j:jb * 8 + j + 1], scale=1.0)
            for tl in range(4):
                tg = pas * 4 + tl
                for n in range(2):
                    ps = ps_f.tile([128, 512], F32, tag="f", name="psf2")
                    for j in range(8):
                        nc.tensor.matmul(
                            out=ps,
                            lhsT=relu_b[:, j, tl * 128:(tl + 1) * 128],
                            rhs=w2b[:, j, n * 512:(n + 1) * 512],
                            start=(j == 0), stop=(j == 7))
                    sl = np.s_[:, n * 512:(n + 1) * 512]
                    if jb == 0:
                        nc.vector.tensor_add(out=oacc[tg][sl],
                                             in0=oacc[tg][sl], in1=ps)
                        nc.vector.tensor_add(out=oacc[tg][sl],
                                             in0=oacc[tg][sl], in1=b2_sb[sl])
                    else:
                        nc.vector.tensor_add(out=oacc[tg][sl],
                                             in0=oacc[tg][sl], in1=ps)
                    if jb == 3:
                        nc.sync.dma_start(
                            out=io["out"][tg * 128:(tg + 1) * 128,
                                          n * 512:(n + 1) * 512],
                            in_=oacc[tg][sl])

    cm_psf.__exit__(None, None, None)
    cm_rl.__exit__(None, None, None)
    cm_w2.__exit__(None, None, None)
    cm_w1.__exit__(None, None, None)
    cm_wp.__exit__(None, None, None)
    cm_at.__exit__(None, None, None)
    cm_ast.__exit__(None, None, None)
    cm_dram.__exit__(None, None, None)
    cm_masks.__exit__(None, None, None)
    cm_v.__exit__(None, None, None)
    cm_qt.__exit__(None, None, None)
    cm_kt.__exit__(None, None, None)
    cm_h1.__exit__(None, None, None)
    cm_small.__exit__(None, None, None)
    cm_stat.__exit__(None, None, None)
    cm_ln.__exit__(None, None, None)
    cm_singles.__exit__(None, None, None)


def build(ln1_triv=True, ln2_triv=True):
    key = (ln1_triv, ln2_triv)
    if key in _CACHE:
        return _CACHE[key]
    nc = bacc.Bacc("TRN2", target_bir_lowering=False, debug=False,
                   num_devices=NCORES)
    io = {}

    def din(name, shape, dt):
        io[name] = nc.dram_tensor(name, list(shape), dt, kind="ExternalInput").ap()

    din("x", (2048, 1024), BF16)
    din("wkh", (128, 8, 1024), BF16)
    din("wvh", (128, 8, 1024), BF16)
    din("wqh", (128, 8, 1024), BF16)
    din("wph", (128, 8, 1024), BF16)
    din("b_proj", (1024,), BF16)
    din("w1h", (4, 128, 8, 1024), BF16)
    din("b1t", (128, 32), F32)
    din("w2h", (4, 128, 8, 1024), BF16)
    din("b2", (1024,), BF16)
    din("masks", (128, 16, 512), BF16)
    if not ln1_triv:
        din("ln1_g", (1024,), F32)
        din("ln1_b", (1024,), F32)
    if not ln2_triv:
        din("ln2_g", (1024,), F32)
        din("ln2_b", (1024,), F32)
    io["out"] = nc.dram_tensor("out", [1024, 1024], F32, kind="ExternalOutput").ap()

    with tile.TileContext(nc) as tc:
        _emit_body(nc, tc, io, ln1_triv, ln2_triv)
    nc.compile()
    _CACHE[key] = (nc, io)
    return nc, io


def _chunks(half):
    # original query-chunk bases (chunk A, chunk B) — rows 0-511 / 512-1023
    # of the per-core output
    if half == 0:
        return (0, 1536)
    return (512, 1024)


def _make_masks(half):
    """[128, 16, 512] bf16; m 0-7 = slot0 (perm k-tiles 0-7 vs slot0
    queries), m 8-15 = slot1 (perm k-tiles 8-15 vs slot1 queries)."""
    P = np.array(TILE_PERM[half])
    tk = np.arange(128)
    ko = P[:, None] * 128 + tk[None, :]                    # [16, 128] orig k idx
    tq = np.arange(512)
    qo0 = (P[tq // 128 + 0] * 128 + tq % 128)              # slot0 queries
    qo1 = (P[tq // 128 + 8] * 128 + tq % 128)              # slot1 queries
    # slot1 skips masks for perm k-tiles 0-7: must be fully allowed
    assert ko[0:8].max() < qo1.min()
    out = np.zeros((128, 16, 512), np.float32)
    for m in range(8):
        out[:, m, :] = (ko[m][:, None] <= qo0[None, :])
    for m in range(8, 16):
        out[:, m, :] = (ko[m][:, None] <= qo1[None, :])
    return out.astype(ml_dtypes.bfloat16)


def _wprep(w):
    # [C, N] f32 -> [128, 8, 1024] bf16 with w_out[p, c, n] = w[c*128+p, n]
    return np.ascontiguousarray(
        w.reshape(8, 128, w.shape[1]).transpose(1, 0, 2)
    ).astype(ml_dtypes.bfloat16)


def _prep_common(inp, ln1_triv, ln2_triv):
    bf = ml_dtypes.bfloat16
    wq_f = np.ascontiguousarray(inp["wq"].transpose(1, 0, 2).reshape(C, C))
    wk_f = np.ascontiguousarray(inp["wk"].transpose(1, 0, 2).reshape(C, C))
    wv_f = np.ascontiguousarray(inp["wv"].transpose(1, 0, 2).reshape(C, C))
    w1 = np.asarray(inp["w1"])
    w2 = np.asarray(inp["w2"])
    # w1h[jb, p, c, n] = w1[c*128+p, jb*1024+n]
    w1h = np.ascontiguousarray(
        w1.reshape(8, 128, 4, 1024).transpose(2, 1, 0, 3)).astype(bf)
    # w2h[jb, p, j, n] = w2[jb*1024 + j*128 + p, n]
    w2h = np.ascontiguousarray(
        w2.reshape(4, 8, 128, 1024).transpose(0, 2, 1, 3)).astype(bf)
    b1t = np.ascontiguousarray(inp["b1"].reshape(32, 128).T).astype(np.float32)
    common = {
        "wqh": _wprep(wq_f),
        "wkh": _wprep(wk_f),
        "wvh": _wprep(wv_f),
        "wph": _wprep(np.asarray(inp["w_proj"])),
        "b_proj": inp["b_proj"].astype(bf),
        "w1h": w1h,
        "b1t": b1t,
        "w2h": w2h,
        "b2": inp["b2"].astype(bf),
    }
    if not ln1_triv:
        common["ln1_g"] = inp["ln1_g"].astype(np.float32)
        common["ln1_b"] = inp["ln1_b"].astype(np.float32)
    if not ln2_triv:
        common["ln2_g"] = inp["ln2_g"].astype(np.float32)
        common["ln2_b"] = inp["ln2_b"].astype(np.float32)
    return common


def make_in_maps(inputs):
    inp = {k: np.asarray(v) for k, v in inputs.items()}
    x = inp["x"].astype(np.float32)
    ln1_triv = bool(np.all(inp["ln1_g"] == 1.0) and np.all(inp["ln1_b"] == 0.0))
    ln2_triv = bool(np.all(inp["ln2_g"] == 1.0) and np.all(inp["ln2_b"] == 0.0))
    common = _prep_common(inp, ln1_triv, ln2_triv)
    in_maps = []
    for i in range(NCORES):
        b, half = i // 2, i % 2
        P = TILE_PERM[half]
        xp = np.ascontiguousarray(
            x[b].reshape(16, 128, C)[P].reshape(2048, C)).astype(
                ml_dtypes.bfloat16)
        m = dict(common)
        m["x"] = xp
        m["masks"] = _make_masks(half)
        in_maps.append(m)
    return in_maps, ln1_triv, ln2_triv


def assemble(results):
    out = np.empty((B, T, C), np.float32)
    for i in range(NCORES):
        b, half = i // 2, i % 2
        qa, qb = _chunks(half)
        o = results[i]["out"]
        out[b, qa:qa + 512] = o[:512]
        out[b, qb:qb + 512] = o[512:]
    return out


def kernel(**inputs):
    in_maps, l1, l2 = make_in_maps(inputs)
    nc, io = build(l1, l2)
    res = run_bass_kernel_spmd(nc, in_maps, list(range(NCORES)))
    return assemble(res.results)


if __name__ == "__main__":
    build()
    print("build ok")

